# revision 25
# baseline (speedup 1.0000x reference)
"""MoE layer (E=8 experts, top-2) on 8 Trainium2 NeuronCores.

Strategy: expert parallelism with host-side routing (the host plays the role
of the all-to-all token dispatch in the sharding hint, exactly like the
host-side combine-sum). Core c holds expert c's weights. The host computes
the router (16 MFLOP), gathers each expert's routed tokens into a fixed
capacity-C buffer, and each core runs the dense FFN over its C tokens:

    yT_c = (gelu(xG_c @ w1_c + b1_c) @ w2_c + b2_c).T     [H, C]

The host scatters the per-core outputs back to token order, scaled by the
top-2 softmax combine weights, and sums the two expert contributions.

Numerics: weights and activations in fp16 (halves weight DMA traffic, which
is otherwise the bottleneck at ~360 GB/s), all matmul accumulation in fp32
PSUM, bias + gelu in fp32 on the scalar engine. Output written in fp32.
"""

import numpy as np

import concourse.mybir as mybir
from concourse import bacc
from concourse.bass_utils import run_bass_kernel_spmd
from concourse.tile import TileContext

FP32 = mybir.dt.float32
FP16 = mybir.dt.float16
AF = mybir.ActivationFunctionType

P = 128
T, H, F, E = 1024, 1024, 4096, 8
HT, FT = H // P, F // P
N_CORES = 8

C_DEFAULT = 272   # expert capacity (max routed load for the fixed input)
NWARM = 7         # PE warmup matmuls to ramp the clock while DMAs land

_cache = {}


def _build_v5(C, act_fn=None):
    act_fn = AF.Gelu if act_fn is None else act_fn
    nc = bacc.Bacc()

    xg = nc.declare_dram_parameter("xg", [P, HT * C], FP16, isOutput=False)
    w1p = nc.declare_dram_parameter("w1p", [P, FT * HT * P], FP16, isOutput=False)
    w2p = nc.declare_dram_parameter("w2p", [P, HT * FT * P], FP16, isOutput=False)
    bias = nc.declare_dram_parameter("bias", [P, FT + HT], FP32, isOutput=False)
    outp = nc.declare_dram_parameter("outp", [H, C], FP32, isOutput=True)

    xg3 = xg.rearrange("p (ht c) -> p ht c", ht=HT)
    w1_4d = w1p.rearrange("p (ft ht fl) -> p ft ht fl", ft=FT, ht=HT)
    w2_4d = w2p.rearrange("p (hh ft hl) -> p hh ft hl", hh=HT, ft=FT)

    # first chunks small so phase-A compute can start early; 2-ft chunks
    # keep DMA supply (~0.73us/ft) ahead of PE demand (~0.91us/ft) with
    # fine availability granularity (each chunk has a +900ns sem lag)
    w1_chunks = [(0, 1), (1, 2), (2, 3), (3, 4)] + [
        (a, min(a + 2, FT)) for a in range(4, FT, 2)
    ]

    with TileContext(nc) as tc:
        with (
            tc.tile_pool(name="const", bufs=1) as const,
            tc.tile_pool(name="wpool", bufs=1) as wpool,
            tc.tile_pool(name="hpool", bufs=1) as hpool,
            tc.tile_pool(name="opool", bufs=3) as opool,
            tc.tile_pool(name="psA", bufs=2, space="PSUM") as psA,
            tc.tile_pool(name="psB", bufs=2, space="PSUM") as psB,
            tc.tile_pool(name="psW", bufs=1, space="PSUM") as psW,
        ):
            # PE warmup: dummy matmuls keep the tensor engine busy (and its
            # p-state ramping) while the first weight/activation DMAs land.
            wmv = const.tile([P, 512], FP16)
            nc.vector.memset(wmv, 0.0)
            pw = psW.tile([P, 512], FP32)
            for _ in range(NWARM):
                nc.tensor.matmul(pw, wmv[:, :P], wmv, start=True, stop=True)

            # DMA issue order == transfer order: first w1 tile + x first so
            # phase-A compute starts as early as possible. The combined bias
            # row (91ns transfer) slots in behind w1 chunk 1 -- it is first
            # needed by the ft0 activation.
            w1sb = wpool.tile([P, FT, HT, P], FP16)
            a, b = w1_chunks[0]
            nc.sync.dma_start(out=w1sb[:, a:b, :, :], in_=w1_4d[:, a:b, :, :])
            xsb = const.tile([P, HT, C], FP16)
            nc.sync.dma_start(out=xsb[:, : HT // 2, :], in_=xg3[:, : HT // 2, :])
            nc.sync.dma_start(out=xsb[:, HT // 2 :, :], in_=xg3[:, HT // 2 :, :])
            bias_sb = const.tile([P, FT + HT], FP32)
            nc.sync.dma_start(out=bias_sb, in_=bias[:, :])
            for a, b in w1_chunks[1:]:
                nc.sync.dma_start(out=w1sb[:, a:b, :, :], in_=w1_4d[:, a:b, :, :])
            w2sb = wpool.tile([P, HT, FT, P], FP16)
            for hh in range(HT):
                nc.sync.dma_start(out=w2sb[:, hh, :, :], in_=w2_4d[:, hh, :, :])

            hG = hpool.tile([P, FT, C], FP16)

            # phase A: hG[f, c] = gelu(w1.T @ xG.T + b1), fp16 out
            for ft in range(FT):
                pa = psA.tile([P, C], FP32, tag="pa", name="pa")
                for ht in range(HT):
                    nc.tensor.matmul(
                        pa,
                        w1sb[:, ft, ht, :],
                        xsb[:, ht, :],
                        start=(ht == 0),
                        stop=(ht == HT - 1),
                    )
                nc.scalar.activation(
                    hG[:, ft, :], pa, act_fn, bias=bias_sb[:, ft : ft + 1]
                )

            # phase B: yT[h', c] = w2.T @ hG + b2, fp32 out to DRAM.
            # The final hh is split into two half-C groups so the last
            # output's DMA pipeline overlaps the closing matmuls.
            CH = C // 2
            spans = [(hh, 0, C) for hh in range(HT - 1)]
            spans += [(HT - 1, 0, CH), (HT - 1, CH, C)]
            for hh, c0, c1 in spans:
                pb = psB.tile([P, c1 - c0], FP32, tag="pb", name="pb")
                for ft in range(FT):
                    nc.tensor.matmul(
                        pb,
                        w2sb[:, hh, ft, :],
                        hG[:, ft, c0:c1],
                        start=(ft == 0),
                        stop=(ft == FT - 1),
                    )
                yt = opool.tile([P, c1 - c0], FP32, tag="yt", name="yt")
                nc.scalar.activation(
                    yt, pb, AF.Identity, bias=bias_sb[:, FT + hh : FT + hh + 1]
                )
                nc.sync.dma_start(out=outp[P * hh : P * (hh + 1), c0:c1], in_=yt)

    nc.compile()
    return nc


def _build_v6(CA, CB, act_fn=None):
    """Expert-pair F-split kernel.

    Experts are paired (heaviest with lightest load); core pair (2i, 2i+1)
    splits the FFN dim F in half, each core computing both experts of pair i
    over half of F. Per-core moving work is CA + CB (max first/second loads
    over pairs) instead of 2*Cmax -- ~3% fewer PE rows -- with identical
    per-core weight DMA volume. The host sums the two half-F partial outputs
    of each pair (b2 is contributed by the half==0 core only).
    """
    act_fn = AF.Gelu if act_fn is None else act_fn
    CS = CA + CB
    FH = FT // 2          # f-tiles per expert on one core (half of F)
    NG = FT               # total phase-A groups: FH per expert * 2 experts

    nc = bacc.Bacc()
    xg = nc.declare_dram_parameter("xg", [P, HT * CS], FP16, isOutput=False)
    w1p = nc.declare_dram_parameter("w1p", [P, NG * HT * P], FP16, isOutput=False)
    w2p = nc.declare_dram_parameter("w2p", [P, HT * NG * P], FP16, isOutput=False)
    bias = nc.declare_dram_parameter("bias", [P, NG + 2 * HT], FP32, isOutput=False)
    outp = nc.declare_dram_parameter("outp", [H, CS], FP32, isOutput=True)

    xg3 = xg.rearrange("p (ht c) -> p ht c", ht=HT)
    w1_4d = w1p.rearrange("p (g ht fl) -> p g ht fl", g=NG, ht=HT)
    w2_4d = w2p.rearrange("p (hh g hl) -> p hh g hl", hh=HT, g=NG)

    w1_chunks = [(0, 2), (2, 3), (3, 4)] + [
        (a, min(a + 2, NG)) for a in range(4, NG, 2)
    ]
    XB_AFTER = 8  # issue the expert-b token load behind this many w1 chunks

    def gcols(g):
        return (0, CA) if g < FH else (CA, CS)

    with TileContext(nc) as tc:
        with (
            tc.tile_pool(name="const", bufs=1) as const,
            tc.tile_pool(name="wpool", bufs=1) as wpool,
            tc.tile_pool(name="hpool", bufs=1) as hpool,
            # one staging buffer per output span: out DMAs queue behind the
            # w2 stream on the DMA device, and a shallow pool would stall
            # the activations (WAR) and back-pressure the PE via PSUM
            tc.tile_pool(name="opool", bufs=2 * HT + 1) as opool,
            tc.tile_pool(name="psA", bufs=2, space="PSUM") as psA,
            tc.tile_pool(name="psB", bufs=2, space="PSUM") as psB,
            tc.tile_pool(name="psW", bufs=1, space="PSUM") as psW,
        ):
            # PE warmup: dummy matmuls keep the tensor engine busy (and its
            # p-state ramping) while the first weight/activation DMAs land.
            wmv = const.tile([P, 512], FP16)
            nc.vector.memset(wmv, 0.0)
            pw = psW.tile([P, 512], FP32)
            for _ in range(NWARM):
                nc.tensor.matmul(pw, wmv[:, :P], wmv, start=True, stop=True)

            # DMA issue order == transfer order: first w1 tile, expert-a
            # tokens, bias; then the w1 stream with the expert-b tokens
            # slotted in partway (needed only when group FH starts).
            w1sb = wpool.tile([P, NG, HT, P], FP16)
            a, b = w1_chunks[0]
            nc.sync.dma_start(out=w1sb[:, a:b, :, :], in_=w1_4d[:, a:b, :, :])
            xsb = const.tile([P, HT, CS], FP16)
            nc.sync.dma_start(
                out=xsb[:, : HT // 2, :CA], in_=xg3[:, : HT // 2, :CA]
            )
            nc.sync.dma_start(
                out=xsb[:, HT // 2 :, :CA], in_=xg3[:, HT // 2 :, :CA]
            )
            bias_sb = const.tile([P, NG + 2 * HT], FP32)
            nc.sync.dma_start(out=bias_sb, in_=bias[:, :])
            for i, (a, b) in enumerate(w1_chunks[1:]):
                nc.sync.dma_start(out=w1sb[:, a:b, :, :], in_=w1_4d[:, a:b, :, :])
                if i == XB_AFTER:
                    nc.sync.dma_start(out=xsb[:, :, CA:], in_=xg3[:, :, CA:])
            w2sb = wpool.tile([P, HT, NG, P], FP16)
            for hh in range(HT):
                nc.sync.dma_start(out=w2sb[:, hh, :, :], in_=w2_4d[:, hh, :, :])

            hG = hpool.tile([P, NG, CA], FP16)

            # phase A: hG[g, c] = gelu(w1.T @ xG.T + b1) for both experts
            for g in range(NG):
                c0, c1 = gcols(g)
                w = c1 - c0
                pa = psA.tile([P, CA], FP32, tag="pa", name="pa")
                for ht in range(HT):
                    nc.tensor.matmul(
                        pa[:, :w],
                        w1sb[:, g, ht, :],
                        xsb[:, ht, c0:c1],
                        start=(ht == 0),
                        stop=(ht == HT - 1),
                    )
                nc.scalar.activation(
                    hG[:, g, :w], pa[:, :w], act_fn, bias=bias_sb[:, g : g + 1]
                )

            # phase B: y_partial[h', c] = w2.T @ hG (+ b2 on half==0 cores).
            # The final group is split in two so the last output's DMA
            # pipeline overlaps the closing matmuls.
            spans = []
            for hh in range(HT):
                spans.append((hh, 0, 0, CA))
                if hh < HT - 1:
                    spans.append((hh, 1, 0, CB))
            spans.append((HT - 1, 1, 0, CB // 2))
            spans.append((HT - 1, 1, CB // 2, CB))
            for hh, ex, c0, c1 in spans:
                w = c1 - c0
                g0 = ex * FH
                pb = psB.tile([P, CA], FP32, tag="pb", name="pb")
                for j in range(FH):
                    nc.tensor.matmul(
                        pb[:, :w],
                        w2sb[:, hh, g0 + j, :],
                        hG[:, g0 + j, c0:c1],
                        start=(j == 0),
                        stop=(j == FH - 1),
                    )
                yt = opool.tile([P, CA], FP32, tag="yt", name="yt")
                bcol = NG + ex * HT + hh
                nc.scalar.activation(
                    yt[:, :w], pb[:, :w], AF.Identity,
                    bias=bias_sb[:, bcol : bcol + 1],
                )
                o0 = ex * CA + c0
                nc.sync.dma_start(
                    out=outp[P * hh : P * (hh + 1), o0 : o0 + w], in_=yt[:, :w]
                )

    nc.compile()
    return nc


def _get_nc(key=None):
    if key is None:
        key = ("v6", 272, 257)
    if key not in _cache:
        if key[0] == "v6":
            _cache[key] = _build_v6(key[1], key[2])
        else:
            _cache[key] = _build_v5(key[1])
    return _cache[key]


def _route(x_flat, gate_w, gate_b):
    """Top-2 routing on host. Returns per-expert (token idx, combine wt)."""
    logits = x_flat @ gate_w.T + gate_b  # (T, E) fp32
    sel = np.argsort(-logits, axis=1, kind="stable")[:, :2]  # (T, 2)
    tw = np.take_along_axis(logits, sel, axis=1)
    tw = tw - tw.max(axis=1, keepdims=True)
    ew = np.exp(tw)
    rw = ew / ew.sum(axis=1, keepdims=True)  # (T, 2)
    idxs, wts = [], []
    for e in range(E):
        m = sel == e  # (T, 2)
        tok = np.nonzero(m.any(axis=1))[0]
        wt = rw[m.any(axis=1), :][m[m.any(axis=1), :]]
        idxs.append(tok)
        wts.append(wt.astype(np.float32))
    return idxs, wts


def kernel(x, gate_w, gate_b, w1, b1, w2, b2):
    x = np.asarray(x, dtype=np.float32)
    gate_w = np.asarray(gate_w, dtype=np.float32)
    gate_b = np.asarray(gate_b, dtype=np.float32)
    w1 = np.asarray(w1, dtype=np.float32)
    b1 = np.asarray(b1, dtype=np.float32)
    w2 = np.asarray(w2, dtype=np.float32)
    b2 = np.asarray(b2, dtype=np.float32)

    x_flat = x.reshape(T, H)
    idxs, wts = _route(x_flat, gate_w, gate_b)
    loads = np.array([len(i) for i in idxs])

    # pair heaviest with lightest load; CA/CB = max first/second pair load
    order = np.argsort(-loads, kind="stable")
    pairs = [(order[i], order[E - 1 - i]) for i in range(E // 2)]
    CA = int(max(loads[a] for a, _ in pairs))
    CB = int(max(loads[b] for _, b in pairs))
    nc = _get_nc(("v6", CA, CB))

    FH = FT // 2
    F2 = F // 2
    NG = FT
    maps = []
    for pi, (ea, eb) in enumerate(pairs):
        xcat = np.zeros((CA + CB, H), dtype=np.float16)
        xcat[: loads[ea]] = x_flat[idxs[ea]]
        xcat[CA : CA + loads[eb]] = x_flat[idxs[eb]]
        xgp = np.ascontiguousarray(
            xcat.reshape(CA + CB, HT, P).transpose(2, 1, 0)
        ).reshape(P, HT * (CA + CB))
        for half in range(2):
            off = half * F2
            w1a = w1[ea][:, off : off + F2].astype(np.float16)  # [H, F2]
            w1b = w1[eb][:, off : off + F2].astype(np.float16)
            w1pk = np.concatenate(
                [
                    w1a.reshape(HT, P, FH, P).transpose(1, 2, 0, 3),
                    w1b.reshape(HT, P, FH, P).transpose(1, 2, 0, 3),
                ],
                axis=1,
            )  # [P, NG, HT, P]
            w2a = w2[ea][off : off + F2, :].astype(np.float16)  # [F2, H]
            w2b = w2[eb][off : off + F2, :].astype(np.float16)
            w2pk = np.concatenate(
                [
                    w2a.reshape(FH, P, HT, P).transpose(1, 2, 0, 3),
                    w2b.reshape(FH, P, HT, P).transpose(1, 2, 0, 3),
                ],
                axis=2,
            )  # [P, HT, NG, P]
            b1cat = np.concatenate(
                [
                    b1[ea][off : off + F2].reshape(FH, P).T,
                    b1[eb][off : off + F2].reshape(FH, P).T,
                ],
                axis=1,
            )  # [P, NG]
            if half == 0:
                b2cat = np.concatenate(
                    [b2[ea].reshape(HT, P).T, b2[eb].reshape(HT, P).T], axis=1
                )
            else:
                b2cat = np.zeros((P, 2 * HT), dtype=np.float32)
            maps.append(
                {
                    "xg": xgp,
                    "w1p": np.ascontiguousarray(w1pk).reshape(P, NG * HT * P),
                    "w2p": np.ascontiguousarray(w2pk).reshape(P, HT * NG * P),
                    "bias": np.ascontiguousarray(
                        np.concatenate([b1cat, b2cat], axis=1)
                    ),
                }
            )

    res = run_bass_kernel_spmd(nc, maps, list(range(N_CORES)))

    out = np.zeros((T, H), dtype=np.float64)
    for pi, (ea, eb) in enumerate(pairs):
        ysum = res.results[2 * pi]["outp"].astype(np.float64) + res.results[
            2 * pi + 1
        ]["outp"].astype(np.float64)  # [H, CA+CB]
        na, nb = loads[ea], loads[eb]
        out[idxs[ea]] += wts[ea][:, None].astype(np.float64) * ysum[:, :na].T
        out[idxs[eb]] += (
            wts[eb][:, None].astype(np.float64) * ysum[:, CA : CA + nb].T
        )
    return out.astype(np.float32).reshape(1, T, H)


# revision 26
# speedup vs baseline: 1.0115x; 1.0115x over previous
"""MoE layer (E=8 experts, top-2) on 8 Trainium2 NeuronCores.

Strategy: expert parallelism with host-side routing (the host plays the role
of the all-to-all token dispatch in the sharding hint, exactly like the
host-side combine-sum). Core c holds expert c's weights. The host computes
the router (16 MFLOP), gathers each expert's routed tokens into a fixed
capacity-C buffer, and each core runs the dense FFN over its C tokens:

    yT_c = (gelu(xG_c @ w1_c + b1_c) @ w2_c + b2_c).T     [H, C]

The host scatters the per-core outputs back to token order, scaled by the
top-2 softmax combine weights, and sums the two expert contributions.

Numerics: weights and activations in fp16 (halves weight DMA traffic, which
is otherwise the bottleneck at ~360 GB/s), all matmul accumulation in fp32
PSUM, bias + gelu in fp32 on the scalar engine. Output written in fp32.
"""

import numpy as np

import concourse.mybir as mybir
from concourse import bacc
from concourse.bass_utils import run_bass_kernel_spmd
from concourse.tile import TileContext

FP32 = mybir.dt.float32
FP16 = mybir.dt.float16
AF = mybir.ActivationFunctionType

P = 128
T, H, F, E = 1024, 1024, 4096, 8
HT, FT = H // P, F // P
N_CORES = 8

C_DEFAULT = 272   # expert capacity (max routed load for the fixed input)
NWARM = 7         # PE warmup matmuls to ramp the clock while DMAs land

_cache = {}


def _build_v5(C, act_fn=None):
    act_fn = AF.Gelu if act_fn is None else act_fn
    nc = bacc.Bacc()

    xg = nc.declare_dram_parameter("xg", [P, HT * C], FP16, isOutput=False)
    w1p = nc.declare_dram_parameter("w1p", [P, FT * HT * P], FP16, isOutput=False)
    w2p = nc.declare_dram_parameter("w2p", [P, HT * FT * P], FP16, isOutput=False)
    bias = nc.declare_dram_parameter("bias", [P, FT + HT], FP32, isOutput=False)
    outp = nc.declare_dram_parameter("outp", [H, C], FP32, isOutput=True)

    xg3 = xg.rearrange("p (ht c) -> p ht c", ht=HT)
    w1_4d = w1p.rearrange("p (ft ht fl) -> p ft ht fl", ft=FT, ht=HT)
    w2_4d = w2p.rearrange("p (hh ft hl) -> p hh ft hl", hh=HT, ft=FT)

    # first chunks small so phase-A compute can start early; 2-ft chunks
    # keep DMA supply (~0.73us/ft) ahead of PE demand (~0.91us/ft) with
    # fine availability granularity (each chunk has a +900ns sem lag)
    w1_chunks = [(0, 1), (1, 2), (2, 3), (3, 4)] + [
        (a, min(a + 2, FT)) for a in range(4, FT, 2)
    ]

    with TileContext(nc) as tc:
        with (
            tc.tile_pool(name="const", bufs=1) as const,
            tc.tile_pool(name="wpool", bufs=1) as wpool,
            tc.tile_pool(name="hpool", bufs=1) as hpool,
            tc.tile_pool(name="opool", bufs=3) as opool,
            tc.tile_pool(name="psA", bufs=2, space="PSUM") as psA,
            tc.tile_pool(name="psB", bufs=2, space="PSUM") as psB,
            tc.tile_pool(name="psW", bufs=1, space="PSUM") as psW,
        ):
            # PE warmup: dummy matmuls keep the tensor engine busy (and its
            # p-state ramping) while the first weight/activation DMAs land.
            wmv = const.tile([P, 512], FP16)
            nc.vector.memset(wmv, 0.0)
            pw = psW.tile([P, 512], FP32)
            for _ in range(NWARM):
                nc.tensor.matmul(pw, wmv[:, :P], wmv, start=True, stop=True)

            # DMA issue order == transfer order: first w1 tile + x first so
            # phase-A compute starts as early as possible. The combined bias
            # row (91ns transfer) slots in behind w1 chunk 1 -- it is first
            # needed by the ft0 activation.
            w1sb = wpool.tile([P, FT, HT, P], FP16)
            a, b = w1_chunks[0]
            nc.sync.dma_start(out=w1sb[:, a:b, :, :], in_=w1_4d[:, a:b, :, :])
            xsb = const.tile([P, HT, C], FP16)
            nc.sync.dma_start(out=xsb[:, : HT // 2, :], in_=xg3[:, : HT // 2, :])
            nc.sync.dma_start(out=xsb[:, HT // 2 :, :], in_=xg3[:, HT // 2 :, :])
            bias_sb = const.tile([P, FT + HT], FP32)
            nc.sync.dma_start(out=bias_sb, in_=bias[:, :])
            for a, b in w1_chunks[1:]:
                nc.sync.dma_start(out=w1sb[:, a:b, :, :], in_=w1_4d[:, a:b, :, :])
            w2sb = wpool.tile([P, HT, FT, P], FP16)
            for hh in range(HT):
                nc.sync.dma_start(out=w2sb[:, hh, :, :], in_=w2_4d[:, hh, :, :])

            hG = hpool.tile([P, FT, C], FP16)

            # phase A: hG[f, c] = gelu(w1.T @ xG.T + b1), fp16 out
            for ft in range(FT):
                pa = psA.tile([P, C], FP32, tag="pa", name="pa")
                for ht in range(HT):
                    nc.tensor.matmul(
                        pa,
                        w1sb[:, ft, ht, :],
                        xsb[:, ht, :],
                        start=(ht == 0),
                        stop=(ht == HT - 1),
                    )
                nc.scalar.activation(
                    hG[:, ft, :], pa, act_fn, bias=bias_sb[:, ft : ft + 1]
                )

            # phase B: yT[h', c] = w2.T @ hG + b2, fp32 out to DRAM.
            # The final hh is split into two half-C groups so the last
            # output's DMA pipeline overlaps the closing matmuls.
            CH = C // 2
            spans = [(hh, 0, C) for hh in range(HT - 1)]
            spans += [(HT - 1, 0, CH), (HT - 1, CH, C)]
            for hh, c0, c1 in spans:
                pb = psB.tile([P, c1 - c0], FP32, tag="pb", name="pb")
                for ft in range(FT):
                    nc.tensor.matmul(
                        pb,
                        w2sb[:, hh, ft, :],
                        hG[:, ft, c0:c1],
                        start=(ft == 0),
                        stop=(ft == FT - 1),
                    )
                yt = opool.tile([P, c1 - c0], FP32, tag="yt", name="yt")
                nc.scalar.activation(
                    yt, pb, AF.Identity, bias=bias_sb[:, FT + hh : FT + hh + 1]
                )
                nc.sync.dma_start(out=outp[P * hh : P * (hh + 1), c0:c1], in_=yt)

    nc.compile()
    return nc


def _build_v6(CA, CB, act_fn=None):
    """Expert-pair F-split kernel.

    Experts are paired (heaviest with lightest load); core pair (2i, 2i+1)
    splits the FFN dim F in half, each core computing both experts of pair i
    over half of F. Per-core moving work is CA + CB (max first/second loads
    over pairs) instead of 2*Cmax -- ~3% fewer PE rows -- with identical
    per-core weight DMA volume. The host sums the two half-F partial outputs
    of each pair (b2 is contributed by the half==0 core only).
    """
    act_fn = AF.Gelu if act_fn is None else act_fn
    CS = CA + CB
    FH = FT // 2          # f-tiles per expert on one core (half of F)
    NG = FT               # total phase-A groups: FH per expert * 2 experts

    nc = bacc.Bacc()
    xg = nc.declare_dram_parameter("xg", [P, HT * CS], FP16, isOutput=False)
    w1p = nc.declare_dram_parameter("w1p", [P, NG * HT * P], FP16, isOutput=False)
    w2p = nc.declare_dram_parameter("w2p", [P, HT * NG * P], FP16, isOutput=False)
    bias = nc.declare_dram_parameter("bias", [P, NG + 2 * HT], FP32, isOutput=False)
    outp = nc.declare_dram_parameter("outp", [H, CS], FP32, isOutput=True)

    xg3 = xg.rearrange("p (ht c) -> p ht c", ht=HT)
    w1_4d = w1p.rearrange("p (g ht fl) -> p g ht fl", g=NG, ht=HT)
    w2_4d = w2p.rearrange("p (hh g hl) -> p hh g hl", hh=HT, g=NG)

    w1_chunks = [(0, 1), (1, 2), (2, 3), (3, 4)] + [
        (a, min(a + 2, NG)) for a in range(4, NG, 2)
    ]
    XB_AFTER = 8  # issue the expert-b token load behind this many w1 chunks

    def gcols(g):
        return (0, CA) if g < FH else (CA, CS)

    with TileContext(nc) as tc:
        with (
            tc.tile_pool(name="const", bufs=1) as const,
            tc.tile_pool(name="wpool", bufs=1) as wpool,
            tc.tile_pool(name="hpool", bufs=1) as hpool,
            # one staging buffer per output span: out DMAs queue behind the
            # w2 stream on the DMA device, and a shallow pool would stall
            # the activations (WAR) and back-pressure the PE via PSUM
            tc.tile_pool(name="opool", bufs=2 * HT + 1) as opool,
            tc.tile_pool(name="psA", bufs=2, space="PSUM") as psA,
            tc.tile_pool(name="psB", bufs=2, space="PSUM") as psB,
            tc.tile_pool(name="psW", bufs=1, space="PSUM") as psW,
        ):
            # PE warmup: dummy matmuls keep the tensor engine busy (and its
            # p-state ramping) while the first weight/activation DMAs land.
            wmv = const.tile([P, 512], FP16)
            nc.vector.memset(wmv, 0.0)
            pw = psW.tile([P, 512], FP32)
            for _ in range(NWARM):
                nc.tensor.matmul(pw, wmv[:, :P], wmv, start=True, stop=True)

            # DMA issue order == transfer order: first w1 tile, expert-a
            # tokens, bias; then the w1 stream with the expert-b tokens
            # slotted in partway (needed only when group FH starts).
            w1sb = wpool.tile([P, NG, HT, P], FP16)
            a, b = w1_chunks[0]
            nc.sync.dma_start(out=w1sb[:, a:b, :, :], in_=w1_4d[:, a:b, :, :])
            xsb = const.tile([P, HT, CS], FP16)
            nc.sync.dma_start(
                out=xsb[:, : HT // 2, :CA], in_=xg3[:, : HT // 2, :CA]
            )
            nc.sync.dma_start(
                out=xsb[:, HT // 2 :, :CA], in_=xg3[:, HT // 2 :, :CA]
            )
            bias_sb = const.tile([P, NG + 2 * HT], FP32)
            nc.sync.dma_start(out=bias_sb, in_=bias[:, :])
            for i, (a, b) in enumerate(w1_chunks[1:]):
                nc.sync.dma_start(out=w1sb[:, a:b, :, :], in_=w1_4d[:, a:b, :, :])
                if i == XB_AFTER:
                    nc.sync.dma_start(out=xsb[:, :, CA:], in_=xg3[:, :, CA:])
            w2sb = wpool.tile([P, HT, NG, P], FP16)
            for hh in range(HT):
                nc.sync.dma_start(out=w2sb[:, hh, :, :], in_=w2_4d[:, hh, :, :])

            hG = hpool.tile([P, NG, CA], FP16)

            # phase A: hG[g, c] = gelu(w1.T @ xG.T + b1) for both experts
            for g in range(NG):
                c0, c1 = gcols(g)
                w = c1 - c0
                pa = psA.tile([P, CA], FP32, tag="pa", name="pa")
                for ht in range(HT):
                    nc.tensor.matmul(
                        pa[:, :w],
                        w1sb[:, g, ht, :],
                        xsb[:, ht, c0:c1],
                        start=(ht == 0),
                        stop=(ht == HT - 1),
                    )
                nc.scalar.activation(
                    hG[:, g, :w], pa[:, :w], act_fn, bias=bias_sb[:, g : g + 1]
                )

            # phase B: y_partial[h', c] = w2.T @ hG (+ b2 on half==0 cores).
            # The final group is split in two so the last output's DMA
            # pipeline overlaps the closing matmuls.
            spans = []
            for hh in range(HT):
                spans.append((hh, 0, 0, CA))
                if hh < HT - 1:
                    spans.append((hh, 1, 0, CB))
            spans.append((HT - 1, 1, 0, CB // 2))
            spans.append((HT - 1, 1, CB // 2, CB))
            for hh, ex, c0, c1 in spans:
                w = c1 - c0
                g0 = ex * FH
                pb = psB.tile([P, CA], FP32, tag="pb", name="pb")
                for j in range(FH):
                    nc.tensor.matmul(
                        pb[:, :w],
                        w2sb[:, hh, g0 + j, :],
                        hG[:, g0 + j, c0:c1],
                        start=(j == 0),
                        stop=(j == FH - 1),
                    )
                yt = opool.tile([P, CA], FP32, tag="yt", name="yt")
                bcol = NG + ex * HT + hh
                nc.scalar.activation(
                    yt[:, :w], pb[:, :w], AF.Identity,
                    bias=bias_sb[:, bcol : bcol + 1],
                )
                o0 = ex * CA + c0
                nc.sync.dma_start(
                    out=outp[P * hh : P * (hh + 1), o0 : o0 + w], in_=yt[:, :w]
                )

    nc.compile()
    return nc


def _get_nc(key=None):
    if key is None:
        key = ("v6", 272, 257)
    if key not in _cache:
        if key[0] == "v6":
            _cache[key] = _build_v6(key[1], key[2])
        else:
            _cache[key] = _build_v5(key[1])
    return _cache[key]


def _route(x_flat, gate_w, gate_b):
    """Top-2 routing on host. Returns per-expert (token idx, combine wt)."""
    logits = x_flat @ gate_w.T + gate_b  # (T, E) fp32
    sel = np.argsort(-logits, axis=1, kind="stable")[:, :2]  # (T, 2)
    tw = np.take_along_axis(logits, sel, axis=1)
    tw = tw - tw.max(axis=1, keepdims=True)
    ew = np.exp(tw)
    rw = ew / ew.sum(axis=1, keepdims=True)  # (T, 2)
    idxs, wts = [], []
    for e in range(E):
        m = sel == e  # (T, 2)
        tok = np.nonzero(m.any(axis=1))[0]
        wt = rw[m.any(axis=1), :][m[m.any(axis=1), :]]
        idxs.append(tok)
        wts.append(wt.astype(np.float32))
    return idxs, wts


def kernel(x, gate_w, gate_b, w1, b1, w2, b2):
    x = np.asarray(x, dtype=np.float32)
    gate_w = np.asarray(gate_w, dtype=np.float32)
    gate_b = np.asarray(gate_b, dtype=np.float32)
    w1 = np.asarray(w1, dtype=np.float32)
    b1 = np.asarray(b1, dtype=np.float32)
    w2 = np.asarray(w2, dtype=np.float32)
    b2 = np.asarray(b2, dtype=np.float32)

    x_flat = x.reshape(T, H)
    idxs, wts = _route(x_flat, gate_w, gate_b)
    loads = np.array([len(i) for i in idxs])

    # pair heaviest with lightest load; CA/CB = max first/second pair load
    order = np.argsort(-loads, kind="stable")
    pairs = [(order[i], order[E - 1 - i]) for i in range(E // 2)]
    CA = int(max(loads[a] for a, _ in pairs))
    CB = int(max(loads[b] for _, b in pairs))
    nc = _get_nc(("v6", CA, CB))

    FH = FT // 2
    F2 = F // 2
    NG = FT
    maps = []
    for pi, (ea, eb) in enumerate(pairs):
        xcat = np.zeros((CA + CB, H), dtype=np.float16)
        xcat[: loads[ea]] = x_flat[idxs[ea]]
        xcat[CA : CA + loads[eb]] = x_flat[idxs[eb]]
        xgp = np.ascontiguousarray(
            xcat.reshape(CA + CB, HT, P).transpose(2, 1, 0)
        ).reshape(P, HT * (CA + CB))
        for half in range(2):
            off = half * F2
            w1a = w1[ea][:, off : off + F2].astype(np.float16)  # [H, F2]
            w1b = w1[eb][:, off : off + F2].astype(np.float16)
            w1pk = np.concatenate(
                [
                    w1a.reshape(HT, P, FH, P).transpose(1, 2, 0, 3),
                    w1b.reshape(HT, P, FH, P).transpose(1, 2, 0, 3),
                ],
                axis=1,
            )  # [P, NG, HT, P]
            w2a = w2[ea][off : off + F2, :].astype(np.float16)  # [F2, H]
            w2b = w2[eb][off : off + F2, :].astype(np.float16)
            w2pk = np.concatenate(
                [
                    w2a.reshape(FH, P, HT, P).transpose(1, 2, 0, 3),
                    w2b.reshape(FH, P, HT, P).transpose(1, 2, 0, 3),
                ],
                axis=2,
            )  # [P, HT, NG, P]
            b1cat = np.concatenate(
                [
                    b1[ea][off : off + F2].reshape(FH, P).T,
                    b1[eb][off : off + F2].reshape(FH, P).T,
                ],
                axis=1,
            )  # [P, NG]
            if half == 0:
                b2cat = np.concatenate(
                    [b2[ea].reshape(HT, P).T, b2[eb].reshape(HT, P).T], axis=1
                )
            else:
                b2cat = np.zeros((P, 2 * HT), dtype=np.float32)
            maps.append(
                {
                    "xg": xgp,
                    "w1p": np.ascontiguousarray(w1pk).reshape(P, NG * HT * P),
                    "w2p": np.ascontiguousarray(w2pk).reshape(P, HT * NG * P),
                    "bias": np.ascontiguousarray(
                        np.concatenate([b1cat, b2cat], axis=1)
                    ),
                }
            )

    res = run_bass_kernel_spmd(nc, maps, list(range(N_CORES)))

    out = np.zeros((T, H), dtype=np.float64)
    for pi, (ea, eb) in enumerate(pairs):
        ysum = res.results[2 * pi]["outp"].astype(np.float64) + res.results[
            2 * pi + 1
        ]["outp"].astype(np.float64)  # [H, CA+CB]
        na, nb = loads[ea], loads[eb]
        out[idxs[ea]] += wts[ea][:, None].astype(np.float64) * ysum[:, :na].T
        out[idxs[eb]] += (
            wts[eb][:, None].astype(np.float64) * ysum[:, CA : CA + nb].T
        )
    return out.astype(np.float32).reshape(1, T, H)
